# revision 1
# baseline (speedup 1.0000x reference)
# Trainium2 Bass kernel for nn_CosSimRouter_pad.
#
# Single fused device program (8 NeuronCores, SPMD, no collectives):
#   The key observation: the pooling weights W depend only on
#   G = normalize(vision) @ normalize(vision).T  — NOT on the (dynamic,
#   host-side) text-score selection. The host computes W for ALL 576
#   candidate rows up front (jnp G + top-16 + softmax, bit-identical to the
#   reference's per-selected-row path), the device pools every candidate
#   row, and the host simply slices the selected rows at the end. This
#   removes the second device-program launch (and its ~20 us of fixed
#   preamble/epilogue/DMA-latency overhead) entirely.
#
#   Device program layout:
#     - pool stage first: out = W @ vision_feature in bf16, column-sharded
#       (512 cols/core). Its matmuls fill the PE-idle window while the big
#       text tensors stream in, and warm the PE p-state.
#     - text stage: cos = vn @ tn.T in bf16 (PE runs bf16 at 1 cyc/row like
#       f32r but DMA bytes halve, so the tensor engine never starves),
#       sharded over the text dim (1024 text rows per core). Each core
#       emits, per vision token and per 512-wide text half, the top-2
#       approximate maxima and their argmax indices; the host rescores all
#       candidates exactly in fp64 and max-combines, so bf16 matmul noise
#       never reaches the (discrete, shape-determining) selection.
#
# Both matmuls keep the contraction dim on partitions; all inputs are laid
# out host-side into [k_tile, 128, free] form so every DMA is contiguous.

import os

os.environ.setdefault("MYCRO_LOCAL_CACHE", "1")

import numpy as np

GAMMA = 0.5
TEMP = 0.05
TOP_K = 16
PAD = 1
GRID = 24
EPS = 1e-8

LV = 576          # vision tokens
LT = 8192         # text tokens
D = 4096          # embed dim
NCORES = 8
LT_SH = LT // NCORES          # 1024 text rows per core
KT = D // 128                 # 32 contraction tiles
NH = 2                        # 512-wide halves of the 1024-wide shard
NCAND = 2                     # device top-2 candidates per (half, token)
M_TILES = (128, 128, 128, 128, 64)   # 576 = 4*128 + 64
KV = 5                        # ceil(576/128) contraction tiles for the pool

_cache: dict = {}


def _build_fused_nc():
    import concourse.mybir as mybir
    import concourse.tile as tile
    from concourse import bacc

    nc = bacc.Bacc(
        "TRN2",
        target_bir_lowering=False,
        debug=False,
        enable_asserts=True,
        num_devices=NCORES,
    )
    bf16 = mybir.dt.bfloat16
    f32 = mybir.dt.float32
    u32 = mybir.dt.uint32
    # partition-major layouts: each SBUF partition's data is one contiguous
    # DRAM run, so chunked DMAs read 16 KB+ per descriptor (near-peak BW)
    vnT = nc.dram_tensor("vnT", [128, KT, LV], bf16, kind="ExternalInput").ap()
    tnT = nc.dram_tensor("tnT", [NH, 128, KT, 512], bf16, kind="ExternalInput").ap()
    wT = nc.dram_tensor("wT", [KV, 128, LV], bf16, kind="ExternalInput").ap()
    vfT = nc.dram_tensor("vfT", [KV, 128, 512], bf16, kind="ExternalInput").ap()
    ident = nc.dram_tensor("ident", [128, 128], f32, kind="ExternalInput").ap()
    # packed argmax results: res[m*8+c, n*128+j] = argmax index (cast to f32)
    # of text half n for vision token m*128+j at rank c. The max VALUES never
    # leave the device — the host rescores every candidate exactly. One
    # 40-partition DMA instead of twenty 128-descriptor strided stores.
    res = nc.dram_tensor("res", [5 * 8, NH * 128], f32, kind="ExternalOutput").ap()
    out = nc.dram_tensor("out", [LV, 512], f32, kind="ExternalOutput").ap()

    # laddered chunk sizes: small first chunks so the first text matmul
    # starts early; big chunks afterwards. The last chunk is processed
    # m-outer so each m-tile's reduction overlaps the remaining matmuls.
    # The second half streams in two coarse chunks (its data isn't needed
    # until ~50 us in; fewer chunks = fewer ring descriptors + sem checks).
    CHUNKS = (1, 1, 2, 2, 4, 6, 8)
    LAST = 8
    CHUNKS1 = (16, 16)
    assert sum(CHUNKS) + LAST == KT
    assert sum(CHUNKS1) == KT

    with tile.TileContext(nc) as tc:
        with (
            tc.tile_pool(name="vn", bufs=1) as vn_pool,
            tc.tile_pool(name="tn", bufs=1) as tn_pool,
            tc.tile_pool(name="w", bufs=1) as w_pool,
            tc.tile_pool(name="vfp", bufs=1) as vf_pool,
            tc.tile_pool(name="red", bufs=1) as red_pool,
            tc.tile_pool(name="ob", bufs=5) as out_pool,
            tc.tile_pool(name="psum", bufs=5, space="PSUM") as psum_pool,
            tc.tile_pool(name="psum2", bufs=3, space="PSUM") as psum2_pool,
        ):
            # text-stage inputs, resident in SBUF (vn 36 KB/part, tn 2x32
            # KB/part), streamed in laddered chunks so compute starts early
            vn_sb = vn_pool.tile([128, KT, LV], bf16)
            tn_sb = [
                tn_pool.tile([128, KT, 512], bf16, name=f"tn_{n}") for n in range(NH)
            ]
            kc = 0
            for ch in CHUNKS + (LAST,):
                nc.scalar.dma_start(vn_sb[:, kc : kc + ch, :], vnT[:, kc : kc + ch, :])
                nc.sync.dma_start(tn_sb[0][:, kc : kc + ch, :], tnT[0, :, kc : kc + ch, :])
                kc += ch
            kc = 0
            for ch in CHUNKS1:
                nc.sync.dma_start(tn_sb[1][:, kc : kc + ch, :], tnT[1, :, kc : kc + ch, :])
                kc += ch
            # pool-stage inputs ride behind the text streams (needed ~90 us in)
            w_sb = w_pool.tile([128, KV, LV], bf16)
            for k in range(KV):
                nc.scalar.dma_start(w_sb[:, k, :], wT[k])
            vf_sb = vf_pool.tile([128, KV, 512], bf16)
            for k in range(KV):
                nc.sync.dma_start(vf_sb[:, k, :], vfT[k])
            id_sb = w_pool.tile([128, 128], f32, name="id_sb")
            nc.scalar.dma_start(id_sb[:, :], ident[:, :])

            # per-half packed argmax accumulators [vision-in-tile, m*8+c];
            # memset because the m=4 tile only fills partitions :64
            mif = [red_pool.tile([128, 40], f32, name=f"mif_{n}") for n in range(NH)]
            for t in mif:
                nc.vector.memset(t[:, :], 0.0)

            # ---- PE p-state warm-up ----
            # The clock sits at 1.2 GHz until ~3 us of continuous execution.
            # The first input chunk only lands ~4 us after the preamble, so
            # burn that idle window on dummy matmuls over a zeroed tile: the
            # PE reaches 2.4 GHz right as the real stream starts.
            warm = red_pool.tile([128, 512], bf16, name="warm")
            nc.vector.memset(warm[:, :], 0.0)
            wps = psum2_pool.tile([128, 512], f32, name="warmps", tag="pps")
            for _ in range(5):
                nc.tensor.matmul(
                    wps[:, :], lhsT=warm[:, 0:128], rhs=warm[:, :],
                    start=True, stop=True,
                )

            # ---- text stage: per-half top-2 of cos over the text shard ----
            for n in range(NH):
                psums = [
                    psum_pool.tile([128, 512], f32, name=f"ps_{n}_{m}", tag="ps")
                    for m in range(len(M_TILES))
                ]
                for k in range(KT - LAST):
                    for m, pm in enumerate(M_TILES):
                        nc.tensor.matmul(
                            psums[m][:pm, :],
                            lhsT=vn_sb[:, k, m * 128 : m * 128 + pm],
                            rhs=tn_sb[n][:, k, :],
                            start=(k == 0),
                            stop=False,
                        )
                # last chunk m-outer: tile m's reduction runs on DVE while
                # tile m+1's matmuls keep the PE busy. max/max_index read the
                # PSUM bank directly — no SBUF staging copy.
                for m, pm in enumerate(M_TILES):
                    for k in range(KT - LAST, KT):
                        nc.tensor.matmul(
                            psums[m][:pm, :],
                            lhsT=vn_sb[:, k, m * 128 : m * 128 + pm],
                            rhs=tn_sb[n][:, k, :],
                            start=False,
                            stop=(k == KT - 1),
                        )
                    mx = red_pool.tile([128, 8], f32, name=f"mx_{n}_{m}")
                    mi = red_pool.tile([128, 8], u32, name=f"mi_{n}_{m}")
                    nc.vector.max(out=mx[:pm, :], in_=psums[m][:pm, :])
                    nc.vector.max_index(
                        out=mi[:pm, :], in_max=mx[:pm, :], in_values=psums[m][:pm, :]
                    )
                    # u32 -> f32 value cast so the PE transpose can move it
                    nc.vector.tensor_copy(mif[n][:pm, m * 8 : (m + 1) * 8], mi[:pm, :])

            # ---- pool stage: out = W @ vf slice, all 576 candidate rows ----
            # Runs on the PE while the DVE drains the n=1 reductions; copies
            # ride the Scalar (activation) engine to keep the DVE free.
            for m, pm in enumerate(M_TILES):
                ps = psum2_pool.tile([128, 512], f32, name=f"pps{m}", tag="pps")
                for k in range(KV):
                    nc.tensor.matmul(
                        ps[:pm, :],
                        lhsT=w_sb[:, k, m * 128 : m * 128 + pm],
                        rhs=vf_sb[:, k, :],
                        start=(k == 0),
                        stop=(k == KV - 1),
                    )
                ot = out_pool.tile([128, 512], f32, name=f"pot{m}", tag="pot")
                nc.scalar.copy(ot[:pm, :], ps[:pm, :])
                # alternate queues so the five output stores drain in parallel
                q = nc.sync if m % 2 == 0 else nc.scalar
                q.dma_start(out[m * 128 : m * 128 + pm, :], ot[:pm, :])

            # ---- pack stage: transpose [128, 40] accumulators to [40, 128]
            # so the result DMA is 40 contiguous 1 KB descriptors ----
            res_sb = red_pool.tile([40, NH * 128], f32, name="res_sb")
            for g, src in enumerate(mif):
                tp = psum2_pool.tile([40, 128], f32, name=f"tp{g}", tag="pps")
                nc.tensor.transpose(tp[:, :], src[:, :], id_sb[:, :])
                nc.scalar.copy(res_sb[:, g * 128 : (g + 1) * 128], tp[:, :])
            # res rides scalar: after the queue split it has the shorter
            # backlog (2 pool outputs vs 3 on sync)
            nc.scalar.dma_start(res[:, :], res_sb[:, :])

    nc.compile()
    return nc


def _get_nc(which: str):
    if which not in _cache:
        _cache[which] = _build_fused_nc()
    return _cache[which]


class _Runner:
    """Cached PJRT executor for one Bass program across the 8 cores.

    Mirrors bass2jax.run_bass_via_pjrt's multi-core branch, but builds the
    jitted shard_map once (that function re-traces and re-compiles on every
    call) and lets chosen inputs be replicated instead of concatenated.

    Call with a dict: sharded inputs as global arrays (axis 0 = n_cores *
    per-core axis 0), replicated inputs at their per-core shape. Returns
    {name: global ndarray} with outputs concatenated along axis 0.
    """

    def __init__(self, nc, replicated=()):
        import jax
        from jax.experimental.shard_map import shard_map
        from jax.sharding import Mesh, PartitionSpec

        import concourse.mybir as mybir
        from concourse import bass2jax

        bass2jax.install_neuronx_cc_hook()
        assert not nc.has_collectives and nc.dbg_addr is None
        self.nc = nc
        part_name = nc.partition_id_tensor.name if nc.partition_id_tensor else None
        in_names, out_names, out_avals = [], [], []
        for alloc in nc.m.functions[0].allocations:
            if not isinstance(alloc, mybir.MemoryLocationSet):
                continue
            name = alloc.memorylocations[0].name
            if alloc.kind == "ExternalInput":
                if name != part_name:
                    in_names.append(name)
            elif alloc.kind == "ExternalOutput":
                out_names.append(name)
                out_avals.append(
                    jax.core.ShapedArray(
                        tuple(alloc.tensor_shape), mybir.dt.np(alloc.dtype)
                    )
                )
        self.in_names, self.out_names, self.out_avals = in_names, out_names, out_avals
        self.replicated = set(replicated)
        n_params = len(in_names)
        donate = tuple(range(n_params, n_params + len(out_names)))

        bind_names = in_names + out_names + ([part_name] if part_name else [])

        def _body(*args):
            operands = list(args)
            if part_name is not None:
                operands.append(bass2jax.partition_id_tensor())
            outs = bass2jax._bass_exec_p.bind(
                *operands,
                out_avals=tuple(out_avals),
                in_names=tuple(bind_names),
                out_names=tuple(out_names),
                lowering_input_output_aliases=(),
                sim_require_finite=True,
                sim_require_nnan=True,
                nc=nc,
            )
            return tuple(outs)

        devices = jax.devices()[:NCORES]
        mesh = Mesh(np.asarray(devices), ("core",))
        in_specs = tuple(
            PartitionSpec() if n in self.replicated else PartitionSpec("core")
            for n in in_names
        ) + (PartitionSpec("core"),) * len(out_names)
        out_specs = (PartitionSpec("core"),) * len(out_names)
        self._fn = jax.jit(
            shard_map(
                _body,
                mesh=mesh,
                in_specs=in_specs,
                out_specs=out_specs,
                check_rep=False,
            ),
            donate_argnums=donate,
            keep_unused=True,
        )

    def __call__(self, inputs: dict):
        args = [np.ascontiguousarray(inputs[n]) for n in self.in_names]
        zeros = [
            np.zeros((NCORES * a.shape[0], *a.shape[1:]), a.dtype)
            for a in self.out_avals
        ]
        outs = self._fn(*args, *zeros)
        return {n: np.asarray(o) for n, o in zip(self.out_names, outs)}


_runners: dict = {}


def _get_runner(which: str) -> _Runner:
    if which not in _runners:
        _runners[which] = _Runner(_get_nc(which), replicated=("vnT", "wT", "ident"))
    return _runners[which]


def _neighbor_unique(sel: np.ndarray) -> np.ndarray:
    offs = np.array(
        [
            [i, j]
            for i in range(-PAD, PAD + 1)
            for j in range(-PAD, PAD + 1)
            if not (i == 0 and j == 0)
        ],
        dtype=np.int64,
    )
    coords = np.stack([sel // GRID, sel % GRID], axis=1)
    padded = np.clip(coords[:, None, :] + offs[None, :, :], 0, GRID - 1)
    return np.unique(padded[..., 0] * GRID + padded[..., 1])


def kernel(vision_feature, text_embed, attention_mask):
    import jax
    import jax.numpy as jnp
    import ml_dtypes

    cpu = jax.devices("cpu")[0]

    vision_feature = np.asarray(vision_feature, dtype=np.float32)
    text_embed = np.asarray(text_embed, dtype=np.float32)
    mask_np = np.asarray(attention_mask)

    with jax.default_device(cpu):
        # normalize exactly as the reference does (jnp on CPU)
        vfj = jnp.asarray(vision_feature)
        tej = jnp.asarray(text_embed)
        vnj = vfj / jnp.maximum(jnp.linalg.norm(vfj, axis=-1, keepdims=True), EPS)
        vn = np.asarray(vnj)
        tn = np.asarray(
            tej / jnp.maximum(jnp.linalg.norm(tej, axis=-1, keepdims=True), EPS)
        )

        # pooling weights for ALL 576 candidate rows. For any row r,
        # (vn @ vn.T)[r] is bit-identical to the reference's
        # normalize(vision[uniq]) @ vn.T row (verified: XLA's row results
        # don't depend on which other rows are present), so top-16 indices
        # and softmax weights match the reference exactly.
        G = vnj @ vnj.T
        top_vals, top_idx = jax.lax.top_k(G, TOP_K)
        w_all = np.asarray(jax.nn.softmax(top_vals, axis=-1))
        top_idx = np.asarray(top_idx)

    W = np.zeros((LV, LV), dtype=np.float32)  # [row r, vision j]
    W[np.arange(LV)[:, None], top_idx] = w_all

    # fold the attention mask into the text rows: where(mask, cos, 0) ==
    # cos * mask elementwise, and max over the text dim commutes with the
    # per-vision positive scale, so pre-scaling text rows by mask is exact.
    tns = tn * mask_np.astype(np.float32)[:, None]

    # ---- device input layouts (all bf16) ----
    vn_bf = vn.astype(ml_dtypes.bfloat16)
    tns_bf = tns.astype(ml_dtypes.bfloat16)
    # vnT[p, k, m] = vn[m, k*128+p]
    vnT = np.ascontiguousarray(vn_bf.T.reshape(KT, 128, LV).transpose(1, 0, 2))
    # global tnT[c*NH+n, p, k, j] = tns[c*1024 + n*512 + j, k*128 + p]
    tnT_g = np.ascontiguousarray(
        tns_bf.reshape(NCORES, NH, 512, KT, 128).transpose(0, 1, 4, 3, 2)
    ).reshape(NCORES * NH, 128, KT, 512)
    WT = np.zeros((KV * 128, LV), dtype=ml_dtypes.bfloat16)
    WT[:LV] = W.T.astype(ml_dtypes.bfloat16)
    wT_r = WT.reshape(KV, 128, LV)  # replicated
    vf_p = np.zeros((KV * 128, D), dtype=ml_dtypes.bfloat16)
    vf_p[:LV] = vision_feature.astype(ml_dtypes.bfloat16)
    # global vfT[c*KV+k, p, j] = vf_p[k*128+p, c*512+j]
    vf_g = np.ascontiguousarray(
        vf_p.reshape(KV, 128, NCORES, 512).transpose(2, 0, 1, 3)
    ).reshape(NCORES * KV, 128, 512)

    out1 = _get_runner("fused")(
        {
            "vnT": vnT,
            "tnT": tnT_g,
            "wT": wT_r,
            "vfT": vf_g,
            "ident": np.eye(128, dtype=np.float32),
        }
    )

    # ---- host: exact rescore of every (core, half, rank) candidate ----
    # res is [NCORES*40, NH*128]: [c, m, rank, n, j] with token = m*128+j
    res = out1["res"].reshape(NCORES, 5, 8, NH, 128)[:, :, :NCAND, :, :]
    amax = (
        res.transpose(0, 3, 2, 1, 4).reshape(NCORES, NH, NCAND, 5 * 128)[:, :, :, :LV]
    ).astype(np.int64)
    n_global = (
        amax
        + np.arange(NCORES)[:, None, None, None] * LT_SH
        + np.arange(NH)[None, :, None, None] * 512
    ).reshape(NCORES * NH * NCAND, LV)
    vn64 = vn.astype(np.float64)
    cand = np.empty((NCORES * NH * NCAND, LV), dtype=np.float64)
    for c in range(cand.shape[0]):
        cand[c] = np.einsum(
            "md,md->m", tns[n_global[c]].astype(np.float64), vn64
        )
    scores = cand.max(axis=0).astype(np.float32)  # [576]

    # ---- host selection (mirrors reference ops; margins >> rescore noise) ----
    with jax.default_device(cpu):
        sj = jnp.asarray(scores)
        probs = jax.nn.softmax(sj / TEMP)
        order = jnp.argsort(-probs)
        cum = jnp.cumsum(probs[order])
        thr = int(jnp.sum(cum <= GAMMA))
        sel = np.asarray(order[:thr])

    if thr == 0:
        return np.zeros((0, D), dtype=np.float32)
    uniq = _neighbor_unique(sel)

    # out is [NCORES*576, 512]: per-core column slices of [576, 4096]
    out_full = (
        out1["out"].reshape(NCORES, LV, 512).transpose(1, 0, 2).reshape(LV, D)
    )
    return np.ascontiguousarray(out_full[uniq])



# revision 8
# speedup vs baseline: 1.7519x; 1.7519x over previous
# Trainium2 Bass kernel for nn_CosSimRouter_pad.
#
# Single fused device program (8 NeuronCores, SPMD, no collectives):
#   The key observation: the pooling weights W depend only on
#   G = normalize(vision) @ normalize(vision).T  — NOT on the (dynamic,
#   host-side) text-score selection. The host computes W for ALL 576
#   candidate rows up front (jnp G + top-16 + softmax, bit-identical to the
#   reference's per-selected-row path), the device pools every candidate
#   row, and the host simply slices the selected rows at the end. This
#   removes the second device-program launch (and its ~20 us of fixed
#   preamble/epilogue/DMA-latency overhead) entirely.
#
#   Device program layout:
#     - pool stage first: out = W @ vision_feature in bf16, column-sharded
#       (512 cols/core). Its matmuls fill the PE-idle window while the big
#       text tensors stream in, and warm the PE p-state.
#     - text stage: cos = vn @ tn.T in fp8 e4m3 with DoubleRow perf mode
#       (2 contraction rows per PE cell per cycle => ~2x bf16 throughput,
#       and half the DMA bytes), sharded over the text dim (1024 text rows
#       per core). Each core emits, per vision token and per 512-wide text
#       half, the top-4 approximate maxima and their argmax indices; the
#       host rescores all candidates exactly in fp64 and max-combines, so
#       fp8 matmul noise (~6e-4 std per score, vs ~5e-3 top-gap) never
#       reaches the (discrete, shape-determining) selection.
#
# Both matmuls keep the contraction dim on partitions; all inputs are laid
# out host-side into [k_tile, 128, free] form so every DMA is contiguous.

import os

os.environ.setdefault("MYCRO_LOCAL_CACHE", "1")

import numpy as np

GAMMA = 0.5
TEMP = 0.05
TOP_K = 16
PAD = 1
GRID = 24
EPS = 1e-8

LV = 576          # vision tokens
LT = 8192         # text tokens
D = 4096          # embed dim
NCORES = 8
LT_SH = LT // NCORES          # 1024 text rows per core
KT = D // 128                 # 32 contraction tiles
NH = 2                        # 512-wide halves of the 1024-wide shard
NCAND = 4                     # device top-4 candidates per (half, token)
M_TILES = (128, 128, 128, 128, 64)   # 576 = 4*128 + 64
KV = 5                        # ceil(576/128) contraction tiles for the pool

_cache: dict = {}


def _build_fused_nc():
    import concourse.mybir as mybir
    import concourse.tile as tile
    from concourse import bacc

    nc = bacc.Bacc(
        "TRN2",
        target_bir_lowering=False,
        debug=False,
        enable_asserts=True,
        num_devices=NCORES,
    )
    bf16 = mybir.dt.bfloat16
    f8 = mybir.dt.float8e4
    f32 = mybir.dt.float32
    u32 = mybir.dt.uint32
    DR = mybir.MatmulPerfMode.DoubleRow
    # partition-major layouts: each SBUF partition's data is one contiguous
    # DRAM run, so chunked DMAs read 16 KB+ per descriptor (near-peak BW)
    vnT = nc.dram_tensor("vnT", [128, KT, LV], f8, kind="ExternalInput").ap()
    tnT = nc.dram_tensor("tnT", [NH, 128, KT, 512], f8, kind="ExternalInput").ap()
    wT = nc.dram_tensor("wT", [KV, 128, LV], bf16, kind="ExternalInput").ap()
    vfT = nc.dram_tensor("vfT", [KV, 128, 512], bf16, kind="ExternalInput").ap()
    ident = nc.dram_tensor("ident", [128, 128], f32, kind="ExternalInput").ap()
    # packed argmax results: res[m*8+c, n*128+j] = argmax index (cast to f32)
    # of text half n for vision token m*128+j at rank c. The max VALUES never
    # leave the device — the host rescores every candidate exactly. One
    # 40-partition DMA instead of twenty 128-descriptor strided stores.
    res = nc.dram_tensor("res", [5 * 8, NH * 128], f32, kind="ExternalOutput").ap()
    out = nc.dram_tensor("out", [LV, 512], f32, kind="ExternalOutput").ap()

    # laddered chunk sizes: small first chunks so the first text matmul
    # starts early; big chunks afterwards. The last chunk is processed
    # m-outer so each m-tile's reduction overlaps the remaining matmuls.
    # The second half streams in two coarse chunks (its data isn't needed
    # until later; fewer chunks = fewer ring descriptors + sem checks).
    # All boundaries even so each DoubleRow matmul's k-tile pair arrives
    # in one chunk.
    CHUNKS = (2, 2, 2, 4, 6, 8)
    LAST = 8
    CHUNKS1 = (16, 16)
    assert sum(CHUNKS) + LAST == KT
    assert sum(CHUNKS1) == KT

    with tile.TileContext(nc) as tc:
        with (
            tc.tile_pool(name="vn", bufs=1) as vn_pool,
            tc.tile_pool(name="tn", bufs=1) as tn_pool,
            tc.tile_pool(name="w", bufs=1) as w_pool,
            tc.tile_pool(name="vfp", bufs=1) as vf_pool,
            tc.tile_pool(name="red", bufs=1) as red_pool,
            tc.tile_pool(name="ob", bufs=5) as out_pool,
            tc.tile_pool(name="psum", bufs=5, space="PSUM") as psum_pool,
            tc.tile_pool(name="psum2", bufs=3, space="PSUM") as psum2_pool,
        ):
            # text-stage inputs, resident in SBUF (fp8: vn 18 KB/part, tn
            # 2x16 KB/part), streamed in laddered chunks so compute starts
            # early
            vn_sb = vn_pool.tile([128, KT, LV], f8)
            tn_sb = [
                tn_pool.tile([128, KT, 512], f8, name=f"tn_{n}") for n in range(NH)
            ]
            kc = 0
            for ch in CHUNKS + (LAST,):
                nc.scalar.dma_start(vn_sb[:, kc : kc + ch, :], vnT[:, kc : kc + ch, :])
                nc.sync.dma_start(tn_sb[0][:, kc : kc + ch, :], tnT[0, :, kc : kc + ch, :])
                kc += ch
            kc = 0
            for ch in CHUNKS1:
                nc.sync.dma_start(tn_sb[1][:, kc : kc + ch, :], tnT[1, :, kc : kc + ch, :])
                kc += ch
            # pool-stage inputs ride behind the text streams (needed ~90 us in)
            w_sb = w_pool.tile([128, KV, LV], bf16)
            for k in range(KV):
                nc.scalar.dma_start(w_sb[:, k, :], wT[k])
            vf_sb = vf_pool.tile([128, KV, 512], bf16)
            for k in range(KV):
                nc.sync.dma_start(vf_sb[:, k, :], vfT[k])
            id_sb = w_pool.tile([128, 128], f32, name="id_sb")
            nc.scalar.dma_start(id_sb[:, :], ident[:, :])

            # per-half packed argmax accumulators [vision-in-tile, m*8+c];
            # memset because the m=4 tile only fills partitions :64
            mif = [red_pool.tile([128, 40], f32, name=f"mif_{n}") for n in range(NH)]
            for t in mif:
                nc.vector.memset(t[:, :], 0.0)

            # ---- PE p-state warm-up ----
            # The clock sits at 1.2 GHz until ~3 us of continuous execution.
            # The first input chunk only lands ~4 us after the preamble, so
            # burn that idle window on dummy matmuls over a zeroed tile: the
            # PE reaches 2.4 GHz right as the real stream starts.
            warm = red_pool.tile([128, 512], bf16, name="warm")
            nc.vector.memset(warm[:, :], 0.0)
            wps = psum2_pool.tile([128, 512], f32, name="warmps", tag="pps")
            for _ in range(5):
                nc.tensor.matmul(
                    wps[:, :], lhsT=warm[:, 0:128], rhs=warm[:, :],
                    start=True, stop=True,
                )

            # ---- text stage: per-half top-8 of cos over the text shard ----
            # fp8 DoubleRow: each matmul consumes TWO 128-row k-tiles
            # (lhsT/rhs sliced [128, 2, free]) at ~0.5 cyc per output column.
            for n in range(NH):
                psums = [
                    psum_pool.tile([128, 512], f32, name=f"ps_{n}_{m}", tag="ps")
                    for m in range(len(M_TILES))
                ]
                for k in range(0, KT - LAST, 2):
                    for m, pm in enumerate(M_TILES):
                        nc.tensor.matmul(
                            psums[m][:pm, :],
                            lhsT=vn_sb[:, k : k + 2, m * 128 : m * 128 + pm],
                            rhs=tn_sb[n][:, k : k + 2, :],
                            start=(k == 0),
                            stop=False,
                            perf_mode=DR,
                        )
                # last chunk m-outer: tile m's reduction runs on DVE while
                # tile m+1's matmuls keep the PE busy. max/max_index read the
                # PSUM bank directly — no SBUF staging copy.
                for m, pm in enumerate(M_TILES):
                    for k in range(KT - LAST, KT, 2):
                        nc.tensor.matmul(
                            psums[m][:pm, :],
                            lhsT=vn_sb[:, k : k + 2, m * 128 : m * 128 + pm],
                            rhs=tn_sb[n][:, k : k + 2, :],
                            start=False,
                            stop=(k == KT - 2),
                            perf_mode=DR,
                        )
                    mx = red_pool.tile([128, 8], f32, name=f"mx_{n}_{m}")
                    mi = red_pool.tile([128, 8], u32, name=f"mi_{n}_{m}")
                    nc.vector.max(out=mx[:pm, :], in_=psums[m][:pm, :])
                    nc.vector.max_index(
                        out=mi[:pm, :], in_max=mx[:pm, :], in_values=psums[m][:pm, :]
                    )
                    # u32 -> f32 value cast so the PE transpose can move it
                    nc.vector.tensor_copy(mif[n][:pm, m * 8 : (m + 1) * 8], mi[:pm, :])

            # ---- pool stage: out = W @ vf slice, all 576 candidate rows ----
            # Runs on the PE while the DVE drains the n=1 reductions; copies
            # ride the Scalar (activation) engine to keep the DVE free.
            for m, pm in enumerate(M_TILES):
                ps = psum2_pool.tile([128, 512], f32, name=f"pps{m}", tag="pps")
                for k in range(KV):
                    nc.tensor.matmul(
                        ps[:pm, :],
                        lhsT=w_sb[:, k, m * 128 : m * 128 + pm],
                        rhs=vf_sb[:, k, :],
                        start=(k == 0),
                        stop=(k == KV - 1),
                    )
                ot = out_pool.tile([128, 512], f32, name=f"pot{m}", tag="pot")
                nc.scalar.copy(ot[:pm, :], ps[:pm, :])
                # alternate queues so the five output stores drain in parallel
                q = nc.sync if m % 2 == 0 else nc.scalar
                q.dma_start(out[m * 128 : m * 128 + pm, :], ot[:pm, :])

            # ---- pack stage: transpose [128, 40] accumulators to [40, 128]
            # so the result DMA is 40 contiguous 1 KB descriptors ----
            res_sb = red_pool.tile([40, NH * 128], f32, name="res_sb")
            for g, src in enumerate(mif):
                tp = psum2_pool.tile([40, 128], f32, name=f"tp{g}", tag="pps")
                nc.tensor.transpose(tp[:, :], src[:, :], id_sb[:, :])
                nc.scalar.copy(res_sb[:, g * 128 : (g + 1) * 128], tp[:, :])
            # res rides scalar: after the queue split it has the shorter
            # backlog (2 pool outputs vs 3 on sync)
            nc.scalar.dma_start(res[:, :], res_sb[:, :])

    nc.compile()
    return nc


def _get_nc(which: str):
    if which not in _cache:
        _cache[which] = _build_fused_nc()
    return _cache[which]


class _Runner:
    """Cached PJRT executor for one Bass program across the 8 cores.

    Mirrors bass2jax.run_bass_via_pjrt's multi-core branch, but builds the
    jitted shard_map once (that function re-traces and re-compiles on every
    call) and lets chosen inputs be replicated instead of concatenated.

    Call with a dict: sharded inputs as global arrays (axis 0 = n_cores *
    per-core axis 0), replicated inputs at their per-core shape. Returns
    {name: global ndarray} with outputs concatenated along axis 0.
    """

    def __init__(self, nc, replicated=()):
        import jax
        from jax.experimental.shard_map import shard_map
        from jax.sharding import Mesh, PartitionSpec

        import concourse.mybir as mybir
        from concourse import bass2jax

        bass2jax.install_neuronx_cc_hook()
        assert not nc.has_collectives and nc.dbg_addr is None
        self.nc = nc
        part_name = nc.partition_id_tensor.name if nc.partition_id_tensor else None
        in_names, out_names, out_avals = [], [], []
        for alloc in nc.m.functions[0].allocations:
            if not isinstance(alloc, mybir.MemoryLocationSet):
                continue
            name = alloc.memorylocations[0].name
            if alloc.kind == "ExternalInput":
                if name != part_name:
                    in_names.append(name)
            elif alloc.kind == "ExternalOutput":
                out_names.append(name)
                out_avals.append(
                    jax.core.ShapedArray(
                        tuple(alloc.tensor_shape), mybir.dt.np(alloc.dtype)
                    )
                )
        self.in_names, self.out_names, self.out_avals = in_names, out_names, out_avals
        self.replicated = set(replicated)
        n_params = len(in_names)
        donate = tuple(range(n_params, n_params + len(out_names)))

        bind_names = in_names + out_names + ([part_name] if part_name else [])

        def _body(*args):
            operands = list(args)
            if part_name is not None:
                operands.append(bass2jax.partition_id_tensor())
            outs = bass2jax._bass_exec_p.bind(
                *operands,
                out_avals=tuple(out_avals),
                in_names=tuple(bind_names),
                out_names=tuple(out_names),
                lowering_input_output_aliases=(),
                sim_require_finite=True,
                sim_require_nnan=True,
                nc=nc,
            )
            return tuple(outs)

        devices = jax.devices()[:NCORES]
        mesh = Mesh(np.asarray(devices), ("core",))
        in_specs = tuple(
            PartitionSpec() if n in self.replicated else PartitionSpec("core")
            for n in in_names
        ) + (PartitionSpec("core"),) * len(out_names)
        out_specs = (PartitionSpec("core"),) * len(out_names)
        self._fn = jax.jit(
            shard_map(
                _body,
                mesh=mesh,
                in_specs=in_specs,
                out_specs=out_specs,
                check_rep=False,
            ),
            donate_argnums=donate,
            keep_unused=True,
        )

    def __call__(self, inputs: dict):
        args = [np.ascontiguousarray(inputs[n]) for n in self.in_names]
        zeros = [
            np.zeros((NCORES * a.shape[0], *a.shape[1:]), a.dtype)
            for a in self.out_avals
        ]
        outs = self._fn(*args, *zeros)
        return {n: np.asarray(o) for n, o in zip(self.out_names, outs)}


_runners: dict = {}


def _get_runner(which: str) -> _Runner:
    if which not in _runners:
        _runners[which] = _Runner(_get_nc(which), replicated=("vnT", "wT", "ident"))
    return _runners[which]


def _neighbor_unique(sel: np.ndarray) -> np.ndarray:
    offs = np.array(
        [
            [i, j]
            for i in range(-PAD, PAD + 1)
            for j in range(-PAD, PAD + 1)
            if not (i == 0 and j == 0)
        ],
        dtype=np.int64,
    )
    coords = np.stack([sel // GRID, sel % GRID], axis=1)
    padded = np.clip(coords[:, None, :] + offs[None, :, :], 0, GRID - 1)
    return np.unique(padded[..., 0] * GRID + padded[..., 1])


def kernel(vision_feature, text_embed, attention_mask):
    import jax
    import jax.numpy as jnp
    import ml_dtypes

    cpu = jax.devices("cpu")[0]

    vision_feature = np.asarray(vision_feature, dtype=np.float32)
    text_embed = np.asarray(text_embed, dtype=np.float32)
    mask_np = np.asarray(attention_mask)

    with jax.default_device(cpu):
        # normalize exactly as the reference does (jnp on CPU)
        vfj = jnp.asarray(vision_feature)
        tej = jnp.asarray(text_embed)
        vnj = vfj / jnp.maximum(jnp.linalg.norm(vfj, axis=-1, keepdims=True), EPS)
        vn = np.asarray(vnj)
        tn = np.asarray(
            tej / jnp.maximum(jnp.linalg.norm(tej, axis=-1, keepdims=True), EPS)
        )

        # pooling weights for ALL 576 candidate rows. For any row r,
        # (vn @ vn.T)[r] is bit-identical to the reference's
        # normalize(vision[uniq]) @ vn.T row (verified: XLA's row results
        # don't depend on which other rows are present), so top-16 indices
        # and softmax weights match the reference exactly.
        G = vnj @ vnj.T
        top_vals, top_idx = jax.lax.top_k(G, TOP_K)
        w_all = np.asarray(jax.nn.softmax(top_vals, axis=-1))
        top_idx = np.asarray(top_idx)

    W = np.zeros((LV, LV), dtype=np.float32)  # [row r, vision j]
    W[np.arange(LV)[:, None], top_idx] = w_all

    # fold the attention mask into the text rows: where(mask, cos, 0) ==
    # cos * mask elementwise, and max over the text dim commutes with the
    # per-vision positive scale, so pre-scaling text rows by mask is exact.
    tns = tn * mask_np.astype(np.float32)[:, None]

    # ---- device input layouts (text stage fp8 e4m3, pool stage bf16) ----
    # TRN float8e4 == ml_dtypes.float8_e4m3 (max 240); our entries are
    # ~N(0, 1/4096) normalized-row values, far inside range.
    vn_f8 = vn.astype(ml_dtypes.float8_e4m3)
    tns_f8 = tns.astype(ml_dtypes.float8_e4m3)
    # vnT[p, k, m] = vn[m, k*128+p]
    vnT = np.ascontiguousarray(vn_f8.T.reshape(KT, 128, LV).transpose(1, 0, 2))
    # global tnT[c*NH+n, p, k, j] = tns[c*1024 + n*512 + j, k*128 + p]
    tnT_g = np.ascontiguousarray(
        tns_f8.reshape(NCORES, NH, 512, KT, 128).transpose(0, 1, 4, 3, 2)
    ).reshape(NCORES * NH, 128, KT, 512)
    WT = np.zeros((KV * 128, LV), dtype=ml_dtypes.bfloat16)
    WT[:LV] = W.T.astype(ml_dtypes.bfloat16)
    wT_r = WT.reshape(KV, 128, LV)  # replicated
    vf_p = np.zeros((KV * 128, D), dtype=ml_dtypes.bfloat16)
    vf_p[:LV] = vision_feature.astype(ml_dtypes.bfloat16)
    # global vfT[c*KV+k, p, j] = vf_p[k*128+p, c*512+j]
    vf_g = np.ascontiguousarray(
        vf_p.reshape(KV, 128, NCORES, 512).transpose(2, 0, 1, 3)
    ).reshape(NCORES * KV, 128, 512)

    out1 = _get_runner("fused")(
        {
            "vnT": vnT,
            "tnT": tnT_g,
            "wT": wT_r,
            "vfT": vf_g,
            "ident": np.eye(128, dtype=np.float32),
        }
    )

    # ---- host: exact rescore of every (core, half, rank) candidate ----
    # res is [NCORES*40, NH*128]: [c, m, rank, n, j] with token = m*128+j
    res = out1["res"].reshape(NCORES, 5, 8, NH, 128)[:, :, :NCAND, :, :]
    amax = (
        res.transpose(0, 3, 2, 1, 4).reshape(NCORES, NH, NCAND, 5 * 128)[:, :, :, :LV]
    ).astype(np.int64)
    n_global = (
        amax
        + np.arange(NCORES)[:, None, None, None] * LT_SH
        + np.arange(NH)[None, :, None, None] * 512
    ).reshape(NCORES * NH * NCAND, LV)
    vn64 = vn.astype(np.float64)
    cand = np.empty((NCORES * NH * NCAND, LV), dtype=np.float64)
    for c in range(cand.shape[0]):
        cand[c] = np.einsum(
            "md,md->m", tns[n_global[c]].astype(np.float64), vn64
        )
    scores = cand.max(axis=0).astype(np.float32)  # [576]

    # ---- host selection (mirrors reference ops; margins >> rescore noise) ----
    with jax.default_device(cpu):
        sj = jnp.asarray(scores)
        probs = jax.nn.softmax(sj / TEMP)
        order = jnp.argsort(-probs)
        cum = jnp.cumsum(probs[order])
        thr = int(jnp.sum(cum <= GAMMA))
        sel = np.asarray(order[:thr])

    if thr == 0:
        return np.zeros((0, D), dtype=np.float32)
    uniq = _neighbor_unique(sel)

    # out is [NCORES*576, 512]: per-core column slices of [576, 4096]
    out_full = (
        out1["out"].reshape(NCORES, LV, 512).transpose(1, 0, 2).reshape(LV, D)
    )
    return np.ascontiguousarray(out_full[uniq])

